# revision 9
# baseline (speedup 1.0000x reference)
"""Trainium2 Bass kernel for nn_CausalFeatureTransformer.

Only the label row (row 128) of the transformer output is returned by the
reference, so the per-node computation collapses to (see kernel_baseline.py
for the derivation):

  zn    = LN(Z[n])                                  (over 128 feats)
  s     = zn / sqrt(zn^2 * vf + eps)                (per feature)
  score = G[h,j] * s[n,j] + D[h,j]                  (label-query attention)
  p     = softmax_j(score)   (max-free: |score| <~ 8)
  num_h = sum_j p*s*Cv_h[j]  den_h = sum_j e        (+ label-token consts)
  x     = (num/den) @ wo + c0
  y     = x + gelu(LN(x) @ w1' + b1') @ w2 + b2

where G, D, Cv, c0, w1', b1', ... are O(params) constants folded on the host.

Sharding: pure data-parallel over nodes N: each of the 8 cores processes a
512-node shard of Z; the folded params are replicated. Device output is
(64, 512) node-major-last; the host transposes on gather.

v2 device-side structure (vs the 2-chunk baseline):
 - full-width 512-node tiles everywhere (fewer instructions, smaller
   event-semaphore cleanup postamble; ACT fixed cost ~300ns/instr amortized)
 - input DMAs spread across 4 engine queues; output across 2
 - all 4 heads in ONE [128,512] PSUM tile (head h at partition base 32h,
   tile_position col bases 0/32/64/96); attention num/den biases and the
   c0 residual are added via rank-1 matmuls that also initialize PSUM
 - per-node mean/var via tensor_reduce sums instead of bn_stats
 - Square / elementwise work split between DVE and the Pool engine
"""

import math

import numpy as np

D_FEAT, D_EMB, H, DK = 128, 64, 4, 16
SEQ = D_FEAT + 1
N = 4096
N_CORES = 8
NS = N // N_CORES  # 512 nodes per core
EPS = 1e-5

_CACHE = {}


def _ln64(x, eps=EPS):
    m = x.mean(-1, keepdims=True)
    v = ((x - m) ** 2).mean(-1, keepdims=True)
    return (x - m) / np.sqrt(v + eps)


def _host_consts(A_full, feat_emb, label_token, wq, bq, wk, bk, wv, bv, wo, bo,
                 w1, b1, w2, b2, alpha, g1, be1, g2, be2):
    """Fold all O(params) quantities on the host (float64 for stability)."""
    import ml_dtypes
    d = np.float64
    fe = feat_emb.astype(d)
    mu = fe.mean(1, keepdims=True)
    vf = ((fe - mu) ** 2).mean(1)                    # (128,)
    cf = (fe - mu) * g1.astype(d)                    # (128,64)

    t = _ln64(label_token.astype(d)[0, 0]) * g1.astype(d) + be1.astype(d)
    qlab = t @ wq.astype(d) + bq.astype(d)
    klab = t @ wk.astype(d) + bk.astype(d)
    vlab = t @ wv.astype(d) + bv.astype(d)

    Ck = cf @ wk.astype(d)                           # (128,64)
    Cv = cf @ wv.astype(d)                           # (128,64)
    bk_p = be1.astype(d) @ wk.astype(d) + bk.astype(d)
    bv_p = be1.astype(d) @ wv.astype(d) + bv.astype(d)

    al = float(alpha)
    rdk = 1.0 / math.sqrt(DK)
    G = np.zeros((H, D_FEAT), d)
    Dm = np.zeros((H, D_FEAT), d)
    slab = np.zeros(H, d)
    for h in range(H):
        blk = slice(h * DK, (h + 1) * DK)
        G[h] = Ck[:, blk] @ qlab[blk] * rdk
        Dm[h] = qlab[blk] @ bk_p[blk] * rdk + al * A_full[:D_FEAT, D_FEAT].astype(d)
        slab[h] = qlab[blk] @ klab[blk] * rdk + al * A_full[D_FEAT, D_FEAT]
    elab = np.exp(slab)                              # (4,)

    c0 = label_token.astype(d)[0, 0] + bv_p @ wo.astype(d) + bo.astype(d)
    w1p = w1.astype(d) * g2.astype(d)[:, None]       # diag(g2) @ w1
    b1p = be2.astype(d) @ w1.astype(d) + b1.astype(d)

    # Head h occupies PSUM partitions [32h, 32h+16) of the shared num tile
    # (den replicated over [32h, 32h+32)).
    nbias = elab[:, None] * (vlab - bv_p).reshape(H, DK)     # (4,16)
    nb = np.zeros(128, d)
    db = np.zeros(128, d)
    cv32 = np.zeros((128, 128), d)
    wo_exp = np.zeros((128, D_EMB), d)
    for h in range(H):
        nb[32 * h:32 * h + DK] = nbias[h]
        db[32 * h:32 * h + 32] = elab[h]
        cv32[:, 32 * h:32 * h + DK] = Cv[:, h * DK:(h + 1) * DK]
        wo_exp[32 * h:32 * h + DK] = wo.astype(d)[h * DK:(h + 1) * DK]

    f32 = np.float32
    bf16 = ml_dtypes.bfloat16

    # fp32 blob (128, 144): vf | b1p | c0+b2 | eps | gcol | dcol | ident
    blob_f = np.zeros((128, 144), f32)
    blob_f[:, 0] = vf
    blob_f[:, 1] = b1p
    blob_f[:64, 2] = c0 + b2
    blob_f[:, 3] = EPS
    blob_f[:, 4:8] = G.T
    blob_f[:, 8:12] = Dm.T
    blob_f[:, 16:144] = np.eye(128, dtype=f32)

    # bf16 blob (128, 1504):
    #  cv32 | w2 | wo_exp | w1p(rows 0:64) | ident | ones32 |
    #  row0: c0row(64) | nb(128) | db(128) | ones512 | ones128
    blob_b = np.zeros((128, 1504), bf16)
    blob_b[:, 0:128] = cv32.astype(bf16)
    blob_b[:, 128:192] = w2.astype(bf16)
    blob_b[:, 192:256] = wo_exp.astype(bf16)
    blob_b[:64, 256:384] = w1p.astype(bf16)
    blob_b[:, 384:512] = np.eye(128, dtype=bf16)
    blob_b[:, 512:544] = bf16(1.0)
    blob_b[0, 544:608] = c0.astype(bf16)
    blob_b[0, 608:736] = nb.astype(bf16)
    blob_b[0, 736:864] = db.astype(bf16)
    blob_b[0, 864:1376] = bf16(1.0)
    blob_b[0, 1376:1504] = bf16(1.0)

    return {"blob_f": blob_f, "blob_b": blob_b}


def _build_bass():
    import concourse.bacc as bacc
    import concourse.mybir as mybir
    import concourse.tile as tile

    f32 = mybir.dt.float32
    bf16 = mybir.dt.bfloat16
    AF = mybir.ActivationFunctionType
    OP = mybir.AluOpType
    AX = mybir.AxisListType

    # Restrict Ln/Exp to the one table set containing both, so the
    # act-table-load pass cannot ping-pong between sets.
    import concourse.hw_specs as hw_specs
    _orig_gat = hw_specs.get_activation_tables

    def _gat(arch):
        t = {k: set(v) for k, v in _orig_gat(arch).items()}
        for name, funcs in t.items():
            if name != "natural_log_exp_and_others":
                funcs.discard(AF.Exp)
                funcs.discard(AF.Ln)
        return t

    bacc.get_activation_tables = _gat

    nc = bacc.Bacc("TRN2", target_bir_lowering=False, debug=False,
                   num_devices=N_CORES)

    zs = nc.dram_tensor("zs", (NS, D_FEAT), f32, kind="ExternalInput")
    blob_f_d = nc.dram_tensor("blob_f", (128, 144), f32, kind="ExternalInput")
    blob_b_d = nc.dram_tensor("blob_b", (128, 1504), bf16, kind="ExternalInput")
    yt = nc.dram_tensor("yt", (D_EMB, NS), f32, kind="ExternalOutput")

    with tile.TileContext(nc) as tc:
        with (
            tc.tile_pool(name="cp", bufs=1) as cp,
            tc.tile_pool(name="wk", bufs=1) as wkp,
            tc.tile_pool(name="ps", bufs=1, space="PSUM") as ps,
        ):
            # ---- input DMAs, one per engine queue
            za = wkp.tile([128, 4, D_FEAT], f32, tag="za", name="za")
            zre = zs.rearrange("(t p) f -> p t f", p=128)
            nc.sync.dma_start(out=za[:, 0:2, :], in_=zre[:, 0:2, :])
            nc.scalar.dma_start(out=za[:, 2:4, :], in_=zre[:, 2:4, :])
            bf = cp.tile([128, 144], f32, tag="bf", name="bf")
            nc.gpsimd.dma_start(out=bf, in_=blob_f_d[:])
            bb = cp.tile([128, 1504], bf16, tag="bb", name="bb")
            nc.gpsimd.dma_start(out=bb, in_=blob_b_d[:])

            vfcol = bf[:, 0:1]
            b1p = bf[:, 1:2]
            c0b2 = bf[:64, 2:3]
            eps_t = bf[:, 3:4]
            gcol = bf[:, 4:8]
            dcol = bf[:, 8:12]
            ident = bf[:, 16:144]
            cv32 = bb[:, 0:128]
            w2m = bb[:, 128:192]
            wo_m = bb[:, 192:256]
            w1p = bb[:64, 256:384]
            identb = bb[:, 384:512]
            ones32 = bb[:, 512:544]
            c0row = bb[0:1, 544:608]
            nb_row = bb[0:1, 608:736]
            db_row = bb[0:1, 736:864]
            ones512 = bb[0:1, 864:1376]
            ones128 = bb[0:1, 1376:1504]

            # ---- LN(Z) stats via sums (Pool computes the squares)
            zasq = wkp.tile([128, 4, D_FEAT], bf16, tag="zasq", name="zasq")
            nc.gpsimd.tensor_tensor(out=zasq, in0=za, in1=za, op=OP.mult)
            sums = wkp.tile([128, 4], f32, tag="sums", name="sums")
            nc.vector.tensor_reduce(out=sums, in_=za, axis=AX.X, op=OP.add)
            sqs = wkp.tile([128, 4], f32, tag="sqs", name="sqs")
            nc.vector.tensor_reduce(out=sqs, in_=zasq, axis=AX.X, op=OP.add)
            m = wkp.tile([128, 4], f32, tag="m", name="m")
            nc.gpsimd.tensor_scalar_mul(out=m, in0=sums, scalar1=1.0 / 128.0)
            sums2 = wkp.tile([128, 4], f32, tag="sums2", name="sums2")
            nc.gpsimd.tensor_tensor(out=sums2, in0=sums, in1=sums, op=OP.mult)
            sq128 = wkp.tile([128, 4], f32, tag="sq128", name="sq128")
            nc.gpsimd.tensor_scalar_mul(out=sq128, in0=sqs, scalar1=128.0)
            vraw = wkp.tile([128, 4], f32, tag="vraw", name="vraw")
            nc.gpsimd.tensor_tensor(out=vraw, in0=sq128, in1=sums2,
                                    op=OP.subtract)
            lnv = wkp.tile([128, 4], f32, tag="lnv", name="lnv")
            nc.scalar.activation(out=lnv, in_=vraw, func=AF.Ln,
                                 scale=1.0 / 16384.0, bias=eps_t)
            rstd = wkp.tile([128, 4], f32, tag="rstd", name="rstd")
            nc.scalar.activation(out=rstd, in_=lnv, func=AF.Exp, scale=-0.5)

            # ---- zn (node-major, bf16), transpose to feat-major PSUM
            zn = wkp.tile([128, 4, D_FEAT], bf16, tag="zn", name="zn")
            for t in range(4):
                nc.vector.tensor_scalar(out=zn[:, t, :], in0=za[:, t, :],
                                        scalar1=m[:, t:t + 1],
                                        scalar2=rstd[:, t:t + 1],
                                        op0=OP.subtract, op1=OP.mult)
            znT = ps.tile([128, NS], bf16, tag="pA", name="znT")
            for t in range(4):
                nc.tensor.transpose(znT[:, t * 128:(t + 1) * 128], zn[:, t, :],
                                    identb)

            # ---- s = zn * rsqrt(zn^2*vf + eps)   (feat-major, full width)
            znTs = wkp.tile([128, NS], bf16, tag="znTs", name="znTs")
            nc.vector.tensor_copy(out=znTs, in_=znT)
            zsq = wkp.tile([128, NS], bf16, tag="zsq", name="zsq")
            nc.gpsimd.tensor_tensor(out=zsq, in0=znTs, in1=znTs, op=OP.mult)
            lnq = wkp.tile([128, NS], f32, tag="lnq", name="lnq")
            nc.scalar.activation(out=lnq, in_=zsq, func=AF.Ln, scale=vfcol,
                                 bias=eps_t)
            rr = wkp.tile([128, NS], f32, tag="rr", name="rr")
            nc.scalar.activation(out=rr, in_=lnq, func=AF.Exp, scale=-0.5)
            sT = wkp.tile([128, NS], bf16, tag="sT", name="sT")
            nc.gpsimd.tensor_mul(out=sT, in0=znTs, in1=rr)

            # ---- attention: all 4 heads in one PSUM tile (base 32h);
            # rank-1 matmuls initialize the tiles with the num/den biases.
            num_ps = ps.tile([128, NS], f32, tag="pB", name="num_ps")
            den_ps = ps.tile([128, NS], f32, tag="pC", name="den_ps")
            nc.tensor.matmul(num_ps, nb_row, ones512, start=True, stop=False)
            nc.tensor.matmul(den_ps, db_row, ones512, start=True, stop=False)
            x_ps = ps.tile([D_EMB, NS], f32, tag="pD", name="x_ps")
            xa_ps = ps.tile([128, 4, D_EMB], f32, tag="pE", name="xa_ps")

            for h in range(4):
                eh = wkp.tile([128, NS], bf16, tag=f"eh{h}", name="eh")
                nc.scalar.activation(out=eh, in_=sT, func=AF.Exp,
                                     scale=gcol[:, h:h + 1],
                                     bias=dcol[:, h:h + 1])
                esh = wkp.tile([128, NS], bf16, tag=f"esh{h}", name="esh")
                eng = nc.vector if h % 2 == 0 else nc.gpsimd
                eng.tensor_mul(out=esh, in0=eh, in1=sT)
                nc.tensor.matmul(den_ps[32 * h:32 * h + 32, :], ones32, eh,
                                 start=False, stop=(h == 3),
                                 tile_position=(0, 32 * h),
                                 skip_group_check=True)
                nc.tensor.matmul(num_ps[32 * h:32 * h + 32, :],
                                 cv32[:, 32 * h:32 * h + 32], esh,
                                 start=False, stop=(h == 3),
                                 tile_position=(0, 32 * h),
                                 skip_group_check=True)

            # ---- oe = num/den; x = wo^T oe (+ c0 via rank-1 init above)
            rcp = wkp.tile([128, NS], f32, tag="rcp", name="rcp")
            nc.vector.reciprocal_approx_fast(out=rcp, in_=den_ps)
            oe = wkp.tile([128, NS], bf16, tag="oe", name="oe")
            nc.vector.tensor_mul(out=oe, in0=num_ps, in1=rcp)
            nc.tensor.matmul(x_ps, wo_m, oe, start=True, stop=False)
            for t in range(4):
                nc.tensor.matmul(xa_ps[:, t, :], ones128, c0row,
                                 start=True, stop=False)
                nc.tensor.matmul(xa_ps[:, t, :],
                                 oe[:, t * 128:(t + 1) * 128], wo_m,
                                 start=False, stop=True)

            # ---- FFN layernorm via sums (xa copied once to SBUF)
            xas = wkp.tile([128, 4, D_EMB], bf16, tag="xas", name="xas")
            nc.vector.tensor_copy(out=xas, in_=xa_ps)
            xasq = wkp.tile([128, 4, D_EMB], bf16, tag="xasq", name="xasq")
            nc.gpsimd.tensor_tensor(out=xasq, in0=xas, in1=xas, op=OP.mult)
            fsums = wkp.tile([128, 4], f32, tag="fsums", name="fsums")
            nc.vector.tensor_reduce(out=fsums, in_=xas, axis=AX.X, op=OP.add)
            fsqs = wkp.tile([128, 4], f32, tag="fsqs", name="fsqs")
            nc.vector.tensor_reduce(out=fsqs, in_=xasq, axis=AX.X, op=OP.add)
            fm = wkp.tile([128, 4], f32, tag="fm", name="fm")
            nc.gpsimd.tensor_scalar_mul(out=fm, in0=fsums, scalar1=1.0 / 64.0)
            fsums2 = wkp.tile([128, 4], f32, tag="fsums2", name="fsums2")
            nc.gpsimd.tensor_tensor(out=fsums2, in0=fsums, in1=fsums,
                                    op=OP.mult)
            fsq64 = wkp.tile([128, 4], f32, tag="fsq64", name="fsq64")
            nc.gpsimd.tensor_scalar_mul(out=fsq64, in0=fsqs, scalar1=64.0)
            fvraw = wkp.tile([128, 4], f32, tag="fvraw", name="fvraw")
            nc.gpsimd.tensor_tensor(out=fvraw, in0=fsq64, in1=fsums2,
                                    op=OP.subtract)
            flnv = wkp.tile([128, 4], f32, tag="flnv", name="flnv")
            nc.scalar.activation(out=flnv, in_=fvraw, func=AF.Ln,
                                 scale=1.0 / 4096.0, bias=eps_t)
            frstd = wkp.tile([128, 4], f32, tag="frstd", name="frstd")
            nc.scalar.activation(out=frstd, in_=flnv, func=AF.Exp, scale=-0.5)

            # ---- u-hat, transpose back to feat-major
            uh = wkp.tile([128, 4, D_EMB], bf16, tag="uh", name="uh")
            for t in range(4):
                nc.vector.tensor_scalar(out=uh[:, t, :], in0=xas[:, t, :],
                                        scalar1=fm[:, t:t + 1],
                                        scalar2=frstd[:, t:t + 1],
                                        op0=OP.subtract, op1=OP.mult)
            uT_ps = ps.tile([D_EMB, NS], bf16, tag="pF", name="uT_ps")
            for t in range(4):
                nc.tensor.transpose(uT_ps[:, t * 128:(t + 1) * 128],
                                    uh[:, t, :], identb)
            uT = wkp.tile([D_EMB, NS], bf16, tag="uT", name="uT")
            nc.vector.tensor_copy(out=uT, in_=uT_ps)

            # ---- FFN matmuls; w2 accumulates into x_ps
            h_ps = ps.tile([2 * D_EMB, NS], f32, tag="pA", name="h_ps")
            nc.tensor.matmul(h_ps, w1p, uT, start=True, stop=True)
            hh = wkp.tile([2 * D_EMB, NS], bf16, tag="hh", name="hh")
            nc.scalar.activation(out=hh, in_=h_ps, func=AF.Gelu, bias=b1p)
            nc.tensor.matmul(x_ps, w2m, hh, start=False, stop=True)

            # ---- y = x + (c0+b2); two halves on two DMA queues
            y0 = wkp.tile([D_EMB, NS // 2], f32, tag="y0", name="y0")
            nc.vector.tensor_scalar_add(out=y0, in0=x_ps[:, 0:NS // 2],
                                        scalar1=c0b2)
            nc.sync.dma_start(out=yt[:, 0:NS // 2], in_=y0)
            y1 = wkp.tile([D_EMB, NS // 2], f32, tag="y1", name="y1")
            nc.scalar.activation(out=y1, in_=x_ps[:, NS // 2:],
                                 func=AF.Identity, bias=c0b2)
            nc.scalar.dma_start(out=yt[:, NS // 2:], in_=y1)

    nc.compile()
    return nc


def _get_nc():
    if "nc" not in _CACHE:
        _CACHE["nc"] = _build_bass()
    return _CACHE["nc"]


def kernel(Z, A_full, feat_emb, label_token, wq, bq, wk, bk, wv, bv, wo, bo,
           w1, b1, w2, b2, alpha, g1, be1, g2, be2, _trace=False,
           _trace_kwargs=None):
    from concourse.bass_utils import run_bass_kernel_spmd

    Z = np.ascontiguousarray(np.asarray(Z, dtype=np.float32))
    consts = _host_consts(
        np.asarray(A_full), np.asarray(feat_emb), np.asarray(label_token),
        np.asarray(wq), np.asarray(bq), np.asarray(wk), np.asarray(bk),
        np.asarray(wv), np.asarray(bv), np.asarray(wo), np.asarray(bo),
        np.asarray(w1), np.asarray(b1), np.asarray(w2), np.asarray(b2),
        np.asarray(alpha), np.asarray(g1), np.asarray(be1), np.asarray(g2),
        np.asarray(be2))
    consts = {k: np.ascontiguousarray(v) for k, v in consts.items()}

    nc = _get_nc()
    in_maps = []
    for c in range(N_CORES):
        mp = dict(consts)
        mp["zs"] = np.ascontiguousarray(Z[c * NS:(c + 1) * NS])
        in_maps.append(mp)

    kw = {}
    if _trace:
        kw["trace"] = True
        if _trace_kwargs:
            kw.update(_trace_kwargs)
    res = run_bass_kernel_spmd(nc, in_maps, core_ids=list(range(N_CORES)), **kw)

    out = np.empty((N, D_EMB), np.float32)
    for c in range(N_CORES):
        out[c * NS:(c + 1) * NS] = res.results[c]["yt"].T
    if _trace:
        return out, res
    return out


# revision 12
# speedup vs baseline: 1.0586x; 1.0586x over previous
"""Trainium2 Bass kernel for nn_CausalFeatureTransformer.

Only the label row (row 128) of the transformer output is returned by the
reference, so the per-node computation collapses to (see kernel_baseline.py
for the derivation):

  zn    = LN(Z[n])                                  (over 128 feats)
  s     = zn / sqrt(zn^2 * vf + eps)                (per feature)
  score = G[h,j] * s[n,j] + D[h,j]                  (label-query attention)
  p     = softmax_j(score)   (max-free: |score| <~ 8)
  num_h = sum_j p*s*Cv_h[j]  den_h = sum_j e        (+ label-token consts)
  x     = (num/den) @ wo + c0
  y     = x + gelu(LN(x) @ w1' + b1') @ w2 + b2

where G, D, Cv, c0, w1', b1', ... are O(params) constants folded on the host.

Sharding: pure data-parallel over nodes N: each of the 8 cores processes a
512-node shard of Z; the folded params are replicated. Device output is
(64, 512) node-major-last; the host transposes on gather.

v2 device-side structure (vs the 2-chunk baseline):
 - full-width 512-node tiles everywhere (fewer instructions, smaller
   event-semaphore cleanup postamble; ACT fixed cost ~300ns/instr amortized)
 - input DMAs spread across 4 engine queues; output across 2
 - all 4 heads in ONE [128,512] PSUM tile (head h at partition base 32h,
   tile_position col bases 0/32/64/96); attention num/den biases and the
   c0 residual are added via rank-1 matmuls that also initialize PSUM
 - per-node mean/var via tensor_reduce sums instead of bn_stats
 - Square / elementwise work split between DVE and the Pool engine
"""

import math

import numpy as np

D_FEAT, D_EMB, H, DK = 128, 64, 4, 16
SEQ = D_FEAT + 1
N = 4096
N_CORES = 8
NS = N // N_CORES  # 512 nodes per core
EPS = 1e-5

_CACHE = {}


def _ln64(x, eps=EPS):
    m = x.mean(-1, keepdims=True)
    v = ((x - m) ** 2).mean(-1, keepdims=True)
    return (x - m) / np.sqrt(v + eps)


def _host_consts(A_full, feat_emb, label_token, wq, bq, wk, bk, wv, bv, wo, bo,
                 w1, b1, w2, b2, alpha, g1, be1, g2, be2):
    """Fold all O(params) quantities on the host (float64 for stability)."""
    import ml_dtypes
    d = np.float64
    fe = feat_emb.astype(d)
    mu = fe.mean(1, keepdims=True)
    vf = ((fe - mu) ** 2).mean(1)                    # (128,)
    cf = (fe - mu) * g1.astype(d)                    # (128,64)

    t = _ln64(label_token.astype(d)[0, 0]) * g1.astype(d) + be1.astype(d)
    qlab = t @ wq.astype(d) + bq.astype(d)
    klab = t @ wk.astype(d) + bk.astype(d)
    vlab = t @ wv.astype(d) + bv.astype(d)

    Ck = cf @ wk.astype(d)                           # (128,64)
    Cv = cf @ wv.astype(d)                           # (128,64)
    bk_p = be1.astype(d) @ wk.astype(d) + bk.astype(d)
    bv_p = be1.astype(d) @ wv.astype(d) + bv.astype(d)

    al = float(alpha)
    rdk = 1.0 / math.sqrt(DK)
    G = np.zeros((H, D_FEAT), d)
    Dm = np.zeros((H, D_FEAT), d)
    slab = np.zeros(H, d)
    for h in range(H):
        blk = slice(h * DK, (h + 1) * DK)
        G[h] = Ck[:, blk] @ qlab[blk] * rdk
        Dm[h] = qlab[blk] @ bk_p[blk] * rdk + al * A_full[:D_FEAT, D_FEAT].astype(d)
        slab[h] = qlab[blk] @ klab[blk] * rdk + al * A_full[D_FEAT, D_FEAT]
    elab = np.exp(slab)                              # (4,)

    c0 = label_token.astype(d)[0, 0] + bv_p @ wo.astype(d) + bo.astype(d)
    w1p = w1.astype(d) * g2.astype(d)[:, None]       # diag(g2) @ w1
    b1p = be2.astype(d) @ w1.astype(d) + b1.astype(d)

    # Head h occupies PSUM partitions [32h, 32h+16) of the shared num tile
    # (den replicated over [32h, 32h+32)).
    nbias = elab[:, None] * (vlab - bv_p).reshape(H, DK)     # (4,16)
    nb = np.zeros(128, d)
    db = np.zeros(128, d)
    cv32 = np.zeros((128, 128), d)
    wo_exp = np.zeros((128, D_EMB), d)
    for h in range(H):
        nb[32 * h:32 * h + DK] = nbias[h]
        db[32 * h:32 * h + 32] = elab[h]
        cv32[:, 32 * h:32 * h + DK] = Cv[:, h * DK:(h + 1) * DK]
        wo_exp[32 * h:32 * h + DK] = wo.astype(d)[h * DK:(h + 1) * DK]

    f32 = np.float32
    bf16 = ml_dtypes.bfloat16

    # fp32 blob (128, 144): vf | b1p | c0+b2 | eps | gcol | dcol | ident
    blob_f = np.zeros((128, 144), f32)
    blob_f[:, 0] = vf
    blob_f[:, 1] = b1p
    blob_f[:64, 2] = c0 + b2
    blob_f[:, 3] = EPS
    blob_f[:, 4:8] = G.T
    blob_f[:, 8:12] = Dm.T
    blob_f[:, 16:144] = np.eye(128, dtype=f32)

    # bf16 blob (128, 1504):
    #  cv32 | w2 | wo_exp | w1p(rows 0:64) | ident | ones32 |
    #  row0: c0row(64) | nb(128) | db(128) | ones512 | ones128
    blob_b = np.zeros((128, 1504), bf16)
    blob_b[:, 0:128] = cv32.astype(bf16)
    blob_b[:, 128:192] = w2.astype(bf16)
    blob_b[:, 192:256] = wo_exp.astype(bf16)
    blob_b[:64, 256:384] = w1p.astype(bf16)
    blob_b[:, 384:512] = np.eye(128, dtype=bf16)
    blob_b[:, 512:544] = bf16(1.0)
    blob_b[0, 544:608] = c0.astype(bf16)
    blob_b[0, 608:736] = nb.astype(bf16)
    blob_b[0, 736:864] = db.astype(bf16)
    blob_b[0, 864:1376] = bf16(1.0)
    blob_b[0, 1376:1504] = bf16(1.0)

    return {"blob_f": blob_f, "blob_b": blob_b}


def _build_bass():
    import concourse.bacc as bacc
    import concourse.mybir as mybir
    import concourse.tile as tile

    f32 = mybir.dt.float32
    bf16 = mybir.dt.bfloat16
    AF = mybir.ActivationFunctionType
    OP = mybir.AluOpType
    AX = mybir.AxisListType

    # Restrict Ln/Exp to the one table set containing both, so the
    # act-table-load pass cannot ping-pong between sets.
    import concourse.hw_specs as hw_specs
    _orig_gat = hw_specs.get_activation_tables

    def _gat(arch):
        t = {k: set(v) for k, v in _orig_gat(arch).items()}
        for name, funcs in t.items():
            if name != "natural_log_exp_and_others":
                funcs.discard(AF.Exp)
                funcs.discard(AF.Ln)
        return t

    bacc.get_activation_tables = _gat

    nc = bacc.Bacc("TRN2", target_bir_lowering=False, debug=False,
                   num_devices=N_CORES)

    zs = nc.dram_tensor("zs", (NS, D_FEAT), f32, kind="ExternalInput")
    blob_f_d = nc.dram_tensor("blob_f", (128, 144), f32, kind="ExternalInput")
    blob_b_d = nc.dram_tensor("blob_b", (128, 1504), bf16, kind="ExternalInput")
    yt = nc.dram_tensor("yt", (D_EMB, NS), f32, kind="ExternalOutput")

    with tile.TileContext(nc) as tc:
        with (
            tc.tile_pool(name="cp", bufs=1) as cp,
            tc.tile_pool(name="wk", bufs=1) as wkp,
            tc.tile_pool(name="ps", bufs=1, space="PSUM") as ps,
        ):
            # ---- input DMAs, one per engine queue
            za = wkp.tile([128, 4, D_FEAT], f32, tag="za", name="za")
            zre = zs.rearrange("(t p) f -> p t f", p=128)
            nc.sync.dma_start(out=za[:, 0:2, :], in_=zre[:, 0:2, :])
            nc.gpsimd.dma_start(out=za[:, 2:4, :], in_=zre[:, 2:4, :])
            bf = cp.tile([128, 144], f32, tag="bf", name="bf")
            nc.sync.dma_start(out=bf, in_=blob_f_d[:])
            bb = cp.tile([128, 1504], bf16, tag="bb", name="bb")
            nc.gpsimd.dma_start(out=bb, in_=blob_b_d[:])

            vfcol = bf[:, 0:1]
            b1p = bf[:, 1:2]
            c0b2 = bf[:64, 2:3]
            eps_t = bf[:, 3:4]
            gcol = bf[:, 4:8]
            dcol = bf[:, 8:12]
            ident = bf[:, 16:144]
            cv32 = bb[:, 0:128]
            w2m = bb[:, 128:192]
            wo_m = bb[:, 192:256]
            w1p = bb[:64, 256:384]
            identb = bb[:, 384:512]
            ones32 = bb[:, 512:544]
            c0row = bb[0:1, 544:608]
            nb_row = bb[0:1, 608:736]
            db_row = bb[0:1, 736:864]
            ones512 = bb[0:1, 864:1376]
            ones128 = bb[0:1, 1376:1504]

            # ---- LN(Z) stats via bn_stats
            mv = wkp.tile([128, 4, 2], f32, tag="mv", name="mv")
            for t in range(4):
                st6 = wkp.tile([128, 6], f32, tag="st6", name="st6")
                nc.vector.bn_stats(out=st6, in_=za[:, t, :])
                nc.vector.bn_aggr(out=mv[:, t, :], in_=st6)
            lnv = wkp.tile([128, 4], f32, tag="lnv", name="lnv")
            nc.scalar.activation(out=lnv, in_=mv[:, :, 1], func=AF.Ln,
                                 bias=eps_t)
            rstd = wkp.tile([128, 4], f32, tag="rstd", name="rstd")
            nc.scalar.activation(out=rstd, in_=lnv, func=AF.Exp, scale=-0.5)

            # ---- zn (node-major, bf16), transpose to feat-major PSUM
            zn = wkp.tile([128, 4, D_FEAT], bf16, tag="zn", name="zn")
            for t in range(4):
                nc.vector.tensor_scalar(out=zn[:, t, :], in0=za[:, t, :],
                                        scalar1=mv[:, t, 0:1],
                                        scalar2=rstd[:, t:t + 1],
                                        op0=OP.subtract, op1=OP.mult)
            znT = ps.tile([128, NS], bf16, tag="pA", name="znT")
            for t in range(4):
                nc.tensor.transpose(znT[:, t * 128:(t + 1) * 128], zn[:, t, :],
                                    identb)

            # ---- s = zn * rsqrt(zn^2*vf + eps)   (feat-major, full width)
            znTs = wkp.tile([128, NS], bf16, tag="znTs", name="znTs")
            nc.vector.tensor_copy(out=znTs, in_=znT)
            zsq = wkp.tile([128, NS], bf16, tag="zsq", name="zsq")
            nc.vector.tensor_tensor(out=zsq, in0=znTs, in1=znTs, op=OP.mult)
            lnq = wkp.tile([128, NS], f32, tag="lnq", name="lnq")
            nc.scalar.activation(out=lnq, in_=zsq, func=AF.Ln, scale=vfcol,
                                 bias=eps_t)
            rr = wkp.tile([128, NS], f32, tag="rr", name="rr")
            nc.scalar.activation(out=rr, in_=lnq, func=AF.Exp, scale=-0.5)
            sT = wkp.tile([128, NS], bf16, tag="sT", name="sT")
            nc.vector.tensor_mul(out=sT, in0=znTs, in1=rr)

            # ---- attention: all 4 heads in one PSUM tile (base 32h);
            # rank-1 matmuls initialize the tiles with the num/den biases.
            num_ps = ps.tile([128, NS], f32, tag="pB", name="num_ps")
            den_ps = ps.tile([128, NS], f32, tag="pC", name="den_ps")
            nc.tensor.matmul(num_ps, nb_row, ones512, start=True, stop=False)
            nc.tensor.matmul(den_ps, db_row, ones512, start=True, stop=False)
            x0_ps = ps.tile([D_EMB, NS // 2], f32, tag="pD", name="x0_ps")
            x1_ps = ps.tile([D_EMB, NS // 2], f32, tag="pH", name="x1_ps")
            xh_ps = [x0_ps, x1_ps]
            xa_ps = ps.tile([128, 4, D_EMB], f32, tag="pE", name="xa_ps")

            for h in range(4):
                eh = wkp.tile([128, NS], bf16, tag=f"eh{h}", name="eh")
                nc.scalar.activation(out=eh, in_=sT, func=AF.Exp,
                                     scale=gcol[:, h:h + 1],
                                     bias=dcol[:, h:h + 1])
                esh = wkp.tile([128, NS], bf16, tag=f"esh{h}", name="esh")
                eng = nc.vector if h == 3 else nc.gpsimd
                eng.tensor_mul(out=esh, in0=eh, in1=sT)
                nc.tensor.matmul(den_ps[32 * h:32 * h + 32, :], ones32, eh,
                                 start=False, stop=(h == 3),
                                 tile_position=(0, 32 * h),
                                 skip_group_check=True)
                nc.tensor.matmul(num_ps[32 * h:32 * h + 32, :],
                                 cv32[:, 32 * h:32 * h + 32], esh,
                                 start=False, stop=(h == 3),
                                 tile_position=(0, 32 * h),
                                 skip_group_check=True)

            # ---- oe = num/den in halves; x = wo^T oe (+ c0 rank-1)
            oeh = []
            for c in range(2):
                sl = slice(c * 256, (c + 1) * 256)
                rcp = wkp.tile([128, 256], f32, tag=f"rcp{c}", name="rcp")
                nc.vector.reciprocal_approx_fast(out=rcp, in_=den_ps[:, sl])
                oe = wkp.tile([128, 256], bf16, tag=f"oe{c}", name="oe")
                nc.vector.tensor_mul(out=oe, in0=num_ps[:, sl], in1=rcp)
                oeh.append(oe)
                nc.tensor.matmul(xh_ps[c], wo_m, oe, start=True, stop=False,
                                 skip_group_check=True)
                for tt_ in range(2):
                    t = 2 * c + tt_
                    nc.tensor.matmul(xa_ps[:, t, :], ones128, c0row,
                                     start=True, stop=False)
                    nc.tensor.matmul(xa_ps[:, t, :],
                                     oe[:, tt_ * 128:(tt_ + 1) * 128], wo_m,
                                     start=False, stop=True)

            # ---- FFN layernorm (bn_stats on PSUM xa), chunked tail
            fmv = wkp.tile([128, 4, 2], f32, tag="fmv", name="fmv")
            for t in range(4):
                fst6 = wkp.tile([128, 6], f32, tag="fst6", name="fst6")
                nc.vector.bn_stats(out=fst6, in_=xa_ps[:, t, :])
                nc.vector.bn_aggr(out=fmv[:, t, :], in_=fst6)
            frstd = wkp.tile([128, 4], f32, tag="frstd", name="frstd")
            for c in range(2):
                flnv = wkp.tile([128, 2], f32, tag=f"flnv{c}", name="flnv")
                nc.scalar.activation(out=flnv, in_=fmv[:, 2 * c:2 * c + 2, 1],
                                     func=AF.Ln, bias=eps_t)
                nc.scalar.activation(out=frstd[:, 2 * c:2 * c + 2], in_=flnv,
                                     func=AF.Exp, scale=-0.5)

            uh = wkp.tile([128, 4, D_EMB], bf16, tag="uh", name="uh")
            uT_ps = ps.tile([D_EMB, NS], bf16, tag="pF", name="uT_ps")
            h_ps = ps.tile([2 * D_EMB, NS], f32, tag="pG", name="h_ps")
            for c in range(2):
                sl = slice(c * 256, (c + 1) * 256)
                for tt_ in range(2):
                    t = 2 * c + tt_
                    nc.vector.tensor_scalar(out=uh[:, t, :],
                                            in0=xa_ps[:, t, :],
                                            scalar1=fmv[:, t, 0:1],
                                            scalar2=frstd[:, t:t + 1],
                                            op0=OP.subtract, op1=OP.mult)
                    nc.tensor.transpose(uT_ps[:, t * 128:(t + 1) * 128],
                                        uh[:, t, :], identb)
                uT = wkp.tile([D_EMB, 256], bf16, tag=f"uT{c}", name="uT")
                nc.vector.tensor_copy(out=uT, in_=uT_ps[:, sl])
                nc.tensor.matmul(h_ps[:, sl], w1p, uT, start=True, stop=True)
                hh = wkp.tile([2 * D_EMB, 256], bf16, tag=f"hh{c}", name="hh")
                nc.scalar.activation(out=hh, in_=h_ps[:, sl], func=AF.Gelu,
                                     bias=b1p)
                nc.tensor.matmul(xh_ps[c], w2m, hh, start=False, stop=True,
                                 skip_group_check=True)

            # ---- y = x + (c0+b2); two halves on two DMA queues
            y0 = wkp.tile([D_EMB, NS // 2], f32, tag="y0", name="y0")
            nc.vector.tensor_scalar_add(out=y0, in0=x0_ps, scalar1=c0b2)
            nc.sync.dma_start(out=yt[:, 0:NS // 2], in_=y0)
            y1 = wkp.tile([D_EMB, NS // 2], f32, tag="y1", name="y1")
            nc.scalar.activation(out=y1, in_=x1_ps,
                                 func=AF.Identity, bias=c0b2)
            nc.scalar.dma_start(out=yt[:, NS // 2:], in_=y1)

    nc.compile()
    return nc


def _get_nc():
    if "nc" not in _CACHE:
        _CACHE["nc"] = _build_bass()
    return _CACHE["nc"]


def kernel(Z, A_full, feat_emb, label_token, wq, bq, wk, bk, wv, bv, wo, bo,
           w1, b1, w2, b2, alpha, g1, be1, g2, be2, _trace=False,
           _trace_kwargs=None):
    from concourse.bass_utils import run_bass_kernel_spmd

    Z = np.ascontiguousarray(np.asarray(Z, dtype=np.float32))
    consts = _host_consts(
        np.asarray(A_full), np.asarray(feat_emb), np.asarray(label_token),
        np.asarray(wq), np.asarray(bq), np.asarray(wk), np.asarray(bk),
        np.asarray(wv), np.asarray(bv), np.asarray(wo), np.asarray(bo),
        np.asarray(w1), np.asarray(b1), np.asarray(w2), np.asarray(b2),
        np.asarray(alpha), np.asarray(g1), np.asarray(be1), np.asarray(g2),
        np.asarray(be2))
    consts = {k: np.ascontiguousarray(v) for k, v in consts.items()}

    nc = _get_nc()
    in_maps = []
    for c in range(N_CORES):
        mp = dict(consts)
        mp["zs"] = np.ascontiguousarray(Z[c * NS:(c + 1) * NS])
        in_maps.append(mp)

    kw = {}
    if _trace:
        kw["trace"] = True
        if _trace_kwargs:
            kw.update(_trace_kwargs)
    res = run_bass_kernel_spmd(nc, in_maps, core_ids=list(range(N_CORES)), **kw)

    out = np.empty((N, D_EMB), np.float32)
    for c in range(N_CORES):
        out[c * NS:(c + 1) * NS] = res.results[c]["yt"].T
    if _trace:
        return out, res
    return out


# revision 14
# speedup vs baseline: 1.1318x; 1.0691x over previous
"""Trainium2 Bass kernel for nn_CausalFeatureTransformer.

Only the label row (row 128) of the transformer output is returned by the
reference, so the per-node computation collapses to (see kernel_baseline.py
for the derivation):

  zn    = LN(Z[n])                                  (over 128 feats)
  s     = zn / sqrt(zn^2 * vf + eps)                (per feature)
  score = G[h,j] * s[n,j] + D[h,j]                  (label-query attention)
  p     = softmax_j(score)   (max-free: |score| <~ 8)
  num_h = sum_j p*s*Cv_h[j]  den_h = sum_j e        (+ label-token consts)
  x     = (num/den) @ wo + c0
  y     = x + gelu(LN(x) @ w1' + b1') @ w2 + b2

where G, D, Cv, c0, w1', b1', ... are O(params) constants folded on the host.

Sharding: pure data-parallel over nodes N: each of the 8 cores processes a
512-node shard of Z; the folded params are replicated. Device output is
(64, 512) node-major-last; the host transposes on gather.

v2 device-side structure (vs the 2-chunk baseline):
 - full-width 512-node tiles everywhere (fewer instructions, smaller
   event-semaphore cleanup postamble; ACT fixed cost ~300ns/instr amortized)
 - input DMAs spread across 4 engine queues; output across 2
 - all 4 heads in ONE [128,512] PSUM tile (head h at partition base 32h,
   tile_position col bases 0/32/64/96); attention num/den biases and the
   c0 residual are added via rank-1 matmuls that also initialize PSUM
 - per-node mean/var via tensor_reduce sums instead of bn_stats
 - Square / elementwise work split between DVE and the Pool engine
"""

import math

import numpy as np

D_FEAT, D_EMB, H, DK = 128, 64, 4, 16
SEQ = D_FEAT + 1
N = 4096
N_CORES = 8
NS = N // N_CORES  # 512 nodes per core
EPS = 1e-5

_CACHE = {}


def _ln64(x, eps=EPS):
    m = x.mean(-1, keepdims=True)
    v = ((x - m) ** 2).mean(-1, keepdims=True)
    return (x - m) / np.sqrt(v + eps)


def _host_consts(A_full, feat_emb, label_token, wq, bq, wk, bk, wv, bv, wo, bo,
                 w1, b1, w2, b2, alpha, g1, be1, g2, be2):
    """Fold all O(params) quantities on the host (float64 for stability)."""
    import ml_dtypes
    d = np.float64
    fe = feat_emb.astype(d)
    mu = fe.mean(1, keepdims=True)
    vf = ((fe - mu) ** 2).mean(1)                    # (128,)
    cf = (fe - mu) * g1.astype(d)                    # (128,64)

    t = _ln64(label_token.astype(d)[0, 0]) * g1.astype(d) + be1.astype(d)
    qlab = t @ wq.astype(d) + bq.astype(d)
    klab = t @ wk.astype(d) + bk.astype(d)
    vlab = t @ wv.astype(d) + bv.astype(d)

    Ck = cf @ wk.astype(d)                           # (128,64)
    Cv = cf @ wv.astype(d)                           # (128,64)
    bk_p = be1.astype(d) @ wk.astype(d) + bk.astype(d)
    bv_p = be1.astype(d) @ wv.astype(d) + bv.astype(d)

    al = float(alpha)
    rdk = 1.0 / math.sqrt(DK)
    G = np.zeros((H, D_FEAT), d)
    Dm = np.zeros((H, D_FEAT), d)
    slab = np.zeros(H, d)
    for h in range(H):
        blk = slice(h * DK, (h + 1) * DK)
        G[h] = Ck[:, blk] @ qlab[blk] * rdk
        Dm[h] = qlab[blk] @ bk_p[blk] * rdk + al * A_full[:D_FEAT, D_FEAT].astype(d)
        slab[h] = qlab[blk] @ klab[blk] * rdk + al * A_full[D_FEAT, D_FEAT]
    elab = np.exp(slab)                              # (4,)

    c0 = label_token.astype(d)[0, 0] + bv_p @ wo.astype(d) + bo.astype(d)
    w1p = w1.astype(d) * g2.astype(d)[:, None]       # diag(g2) @ w1
    b1p = be2.astype(d) @ w1.astype(d) + b1.astype(d)

    # Head h occupies PSUM partitions [32h, 32h+16) of the shared num tile
    # (den replicated over [32h, 32h+32)).
    nbias = elab[:, None] * (vlab - bv_p).reshape(H, DK)     # (4,16)
    nb = np.zeros(128, d)
    db = np.zeros(128, d)
    cv32 = np.zeros((128, 128), d)
    wo_exp = np.zeros((128, D_EMB), d)
    for h in range(H):
        nb[32 * h:32 * h + DK] = nbias[h]
        db[32 * h:32 * h + 32] = elab[h]
        cv32[:, 32 * h:32 * h + DK] = Cv[:, h * DK:(h + 1) * DK]
        wo_exp[32 * h:32 * h + DK] = wo.astype(d)[h * DK:(h + 1) * DK]

    f32 = np.float32
    bf16 = ml_dtypes.bfloat16

    # fp32 blob (128, 144): vf | b1p | c0+b2 | eps | gcol | dcol | ident
    blob_f = np.zeros((128, 144), f32)
    blob_f[:, 0] = vf
    blob_f[:, 1] = b1p
    blob_f[:64, 2] = c0 + b2
    blob_f[:, 3] = EPS
    blob_f[:, 4:8] = G.T
    blob_f[:, 8:12] = Dm.T
    blob_f[:, 16:144] = np.eye(128, dtype=f32)

    # bf16 blob (128, 1504):
    #  cv32 | w2 | wo_exp | w1p(rows 0:64) | ident | ones32 |
    #  row0: c0row(64) | nb(128) | db(128) | ones512 | ones128
    blob_b = np.zeros((128, 1504), bf16)
    blob_b[:, 0:128] = cv32.astype(bf16)
    blob_b[:, 128:192] = w2.astype(bf16)
    blob_b[:, 192:256] = wo_exp.astype(bf16)
    blob_b[:64, 256:384] = w1p.astype(bf16)
    blob_b[:, 384:512] = np.eye(128, dtype=bf16)
    blob_b[:, 512:544] = bf16(1.0)
    blob_b[0, 544:608] = c0.astype(bf16)
    blob_b[0, 608:736] = nb.astype(bf16)
    blob_b[0, 736:864] = db.astype(bf16)
    blob_b[0, 864:1376] = bf16(1.0)
    blob_b[0, 1376:1504] = bf16(1.0)

    return {"blob_f": blob_f, "blob_b": blob_b}


def _build_bass():
    import concourse.bacc as bacc
    import concourse.mybir as mybir
    import concourse.tile as tile

    f32 = mybir.dt.float32
    bf16 = mybir.dt.bfloat16
    AF = mybir.ActivationFunctionType
    OP = mybir.AluOpType
    AX = mybir.AxisListType

    # Restrict Ln/Exp to the one table set containing both, so the
    # act-table-load pass cannot ping-pong between sets.
    import concourse.hw_specs as hw_specs
    _orig_gat = hw_specs.get_activation_tables

    def _gat(arch):
        t = {k: set(v) for k, v in _orig_gat(arch).items()}
        for name, funcs in t.items():
            if name != "natural_log_exp_and_others":
                funcs.discard(AF.Exp)
                funcs.discard(AF.Ln)
        return t

    bacc.get_activation_tables = _gat

    nc = bacc.Bacc("TRN2", target_bir_lowering=False, debug=False,
                   num_devices=N_CORES)

    zs = nc.dram_tensor("zs", (NS, D_FEAT), f32, kind="ExternalInput")
    blob_f_d = nc.dram_tensor("blob_f", (128, 144), f32, kind="ExternalInput")
    blob_b_d = nc.dram_tensor("blob_b", (128, 1504), bf16, kind="ExternalInput")
    yt = nc.dram_tensor("yt", (D_EMB, NS), f32, kind="ExternalOutput")

    with tile.TileContext(nc) as tc:
        with (
            tc.tile_pool(name="cp", bufs=1) as cp,
            tc.tile_pool(name="wk", bufs=1) as wkp,
            tc.tile_pool(name="ps", bufs=1, space="PSUM") as ps,
        ):
            # ---- input DMAs, one per engine queue
            za = wkp.tile([128, 4, D_FEAT], f32, tag="za", name="za")
            zre = zs.rearrange("(t p) f -> p t f", p=128)
            nc.sync.dma_start(out=za[:, 0:2, :], in_=zre[:, 0:2, :])
            nc.gpsimd.dma_start(out=za[:, 2:4, :], in_=zre[:, 2:4, :])
            bf = cp.tile([128, 144], f32, tag="bf", name="bf")
            nc.sync.dma_start(out=bf, in_=blob_f_d[:])
            bb = cp.tile([128, 1504], bf16, tag="bb", name="bb")
            nc.gpsimd.dma_start(out=bb, in_=blob_b_d[:])

            vfcol = bf[:, 0:1]
            b1p = bf[:, 1:2]
            c0b2 = bf[:64, 2:3]
            eps_t = bf[:, 3:4]
            gcol = bf[:, 4:8]
            dcol = bf[:, 8:12]
            ident = bf[:, 16:144]
            cv32 = bb[:, 0:128]
            w2m = bb[:, 128:192]
            wo_m = bb[:, 192:256]
            w1p = bb[:64, 256:384]
            identb = bb[:, 384:512]
            ones32 = bb[:, 512:544]
            c0row = bb[0:1, 544:608]
            nb_row = bb[0:1, 608:736]
            db_row = bb[0:1, 736:864]
            ones512 = bb[0:1, 864:1376]
            ones128 = bb[0:1, 1376:1504]

            # dummy Ln on a const tile: places the Ln/Exp ACT_TABLE_LOAD
            # at the top of the scalar stream, overlapping the input DMAs
            dum = cp.tile([1, 1], f32, tag="dum", name="dum")
            nc.gpsimd.memset(dum, 1.0)
            dml = cp.tile([1, 1], f32, tag="dml", name="dml")
            nc.scalar.activation(out=dml, in_=dum, func=AF.Ln, bias=0.0)

            # ---- LN(Z) stats via bn_stats
            mv = wkp.tile([128, 4, 2], f32, tag="mv", name="mv")
            rstd = wkp.tile([128, 4], f32, tag="rstd", name="rstd")
            for c in range(2):
                for t in (2 * c, 2 * c + 1):
                    st6 = wkp.tile([128, 6], f32, tag="st6", name="st6")
                    nc.vector.bn_stats(out=st6, in_=za[:, t, :])
                    nc.vector.bn_aggr(out=mv[:, t, :], in_=st6)
                lnv = wkp.tile([128, 2], f32, tag=f"lnv{c}", name="lnv")
                nc.scalar.activation(out=lnv, in_=mv[:, 2 * c:2 * c + 2, 1],
                                     func=AF.Ln, bias=eps_t)
                nc.scalar.activation(out=rstd[:, 2 * c:2 * c + 2], in_=lnv,
                                     func=AF.Exp, scale=-0.5)

            # ---- zn (node-major, bf16), transpose to feat-major PSUM
            zn = wkp.tile([128, 4, D_FEAT], bf16, tag="zn", name="zn")
            for t in range(4):
                nc.vector.tensor_scalar(out=zn[:, t, :], in0=za[:, t, :],
                                        scalar1=mv[:, t, 0:1],
                                        scalar2=rstd[:, t:t + 1],
                                        op0=OP.subtract, op1=OP.mult)
            znT = ps.tile([128, NS], bf16, tag="pA", name="znT")
            for t in range(4):
                nc.tensor.transpose(znT[:, t * 128:(t + 1) * 128], zn[:, t, :],
                                    identb)

            # ---- s = zn * rsqrt(zn^2*vf + eps)   (feat-major, full width)
            znTs = wkp.tile([128, NS], bf16, tag="znTs", name="znTs")
            nc.vector.tensor_copy(out=znTs, in_=znT)
            zsq = wkp.tile([128, NS], bf16, tag="zsq", name="zsq")
            nc.vector.tensor_tensor(out=zsq, in0=znTs, in1=znTs, op=OP.mult)
            lnq = wkp.tile([128, NS], f32, tag="lnq", name="lnq")
            nc.scalar.activation(out=lnq, in_=zsq, func=AF.Ln, scale=vfcol,
                                 bias=eps_t)
            rr = wkp.tile([128, NS], f32, tag="rr", name="rr")
            nc.scalar.activation(out=rr, in_=lnq, func=AF.Exp, scale=-0.5)
            sT = wkp.tile([128, NS], bf16, tag="sT", name="sT")
            nc.vector.tensor_mul(out=sT, in0=znTs, in1=rr)

            # ---- attention: all 4 heads in one PSUM tile (base 32h);
            # rank-1 matmuls initialize the tiles with the num/den biases.
            num_ps = ps.tile([128, NS], f32, tag="pB", name="num_ps")
            den_ps = ps.tile([128, NS], f32, tag="pC", name="den_ps")
            nc.tensor.matmul(num_ps, nb_row, ones512, start=True, stop=False)
            nc.tensor.matmul(den_ps, db_row, ones512, start=True, stop=False)
            x0_ps = ps.tile([D_EMB, NS // 2], f32, tag="pD", name="x0_ps")
            x1_ps = ps.tile([D_EMB, NS // 2], f32, tag="pH", name="x1_ps")
            xh_ps = [x0_ps, x1_ps]
            xa_ps = ps.tile([128, 4, D_EMB], f32, tag="pE", name="xa_ps")

            for h in range(4):
                eh = wkp.tile([128, NS], bf16, tag=f"eh{h}", name="eh")
                nc.scalar.activation(out=eh, in_=sT, func=AF.Exp,
                                     scale=gcol[:, h:h + 1],
                                     bias=dcol[:, h:h + 1])
                esh = wkp.tile([128, NS], bf16, tag=f"esh{h}", name="esh")
                nc.vector.tensor_mul(out=esh, in0=eh, in1=sT)
                nc.tensor.matmul(den_ps[32 * h:32 * h + 32, :], ones32, eh,
                                 start=False, stop=(h == 3),
                                 tile_position=(0, 32 * h),
                                 skip_group_check=True)
                nc.tensor.matmul(num_ps[32 * h:32 * h + 32, :],
                                 cv32[:, 32 * h:32 * h + 32], esh,
                                 start=False, stop=(h == 3),
                                 tile_position=(0, 32 * h),
                                 skip_group_check=True)

            # ---- oe = num/den in halves; x = wo^T oe (+ c0 rank-1)
            oeh = []
            for c in range(2):
                sl = slice(c * 256, (c + 1) * 256)
                rcp = wkp.tile([128, 256], f32, tag=f"rcp{c}", name="rcp")
                nc.vector.reciprocal_approx_fast(out=rcp, in_=den_ps[:, sl])
                oe = wkp.tile([128, 256], bf16, tag=f"oe{c}", name="oe")
                nc.vector.tensor_mul(out=oe, in0=num_ps[:, sl], in1=rcp)
                oeh.append(oe)
                nc.tensor.matmul(xh_ps[c], wo_m, oe, start=True, stop=False,
                                 skip_group_check=True)
                for tt_ in range(2):
                    t = 2 * c + tt_
                    nc.tensor.matmul(xa_ps[:, t, :], ones128, c0row,
                                     start=True, stop=False)
                    nc.tensor.matmul(xa_ps[:, t, :],
                                     oe[:, tt_ * 128:(tt_ + 1) * 128], wo_m,
                                     start=False, stop=True)

            # ---- FFN layernorm (bn_stats on PSUM xa), chunked tail
            fmv = wkp.tile([128, 4, 2], f32, tag="fmv", name="fmv")
            for t in range(4):
                fst6 = wkp.tile([128, 6], f32, tag="fst6", name="fst6")
                nc.vector.bn_stats(out=fst6, in_=xa_ps[:, t, :])
                nc.vector.bn_aggr(out=fmv[:, t, :], in_=fst6)
            frstd = wkp.tile([128, 4], f32, tag="frstd", name="frstd")
            for c in range(2):
                flnv = wkp.tile([128, 2], f32, tag=f"flnv{c}", name="flnv")
                nc.scalar.activation(out=flnv, in_=fmv[:, 2 * c:2 * c + 2, 1],
                                     func=AF.Ln, bias=eps_t)
                nc.scalar.activation(out=frstd[:, 2 * c:2 * c + 2], in_=flnv,
                                     func=AF.Exp, scale=-0.5)

            uh = wkp.tile([128, 4, D_EMB], bf16, tag="uh", name="uh")
            uT_ps = ps.tile([D_EMB, NS], bf16, tag="pF", name="uT_ps")
            h_ps = ps.tile([2 * D_EMB, NS], f32, tag="pG", name="h_ps")
            for c in range(2):
                sl = slice(c * 256, (c + 1) * 256)
                for tt_ in range(2):
                    t = 2 * c + tt_
                    nc.vector.tensor_scalar(out=uh[:, t, :],
                                            in0=xa_ps[:, t, :],
                                            scalar1=fmv[:, t, 0:1],
                                            scalar2=frstd[:, t:t + 1],
                                            op0=OP.subtract, op1=OP.mult)
                    nc.tensor.transpose(uT_ps[:, t * 128:(t + 1) * 128],
                                        uh[:, t, :], identb)
                uT = wkp.tile([D_EMB, 256], bf16, tag=f"uT{c}", name="uT")
                nc.vector.tensor_copy(out=uT, in_=uT_ps[:, sl])
                nc.tensor.matmul(h_ps[:, sl], w1p, uT, start=True, stop=True)
                hh = wkp.tile([2 * D_EMB, 256], bf16, tag=f"hh{c}", name="hh")
                nc.scalar.activation(out=hh, in_=h_ps[:, sl], func=AF.Gelu,
                                     bias=b1p)
                nc.tensor.matmul(xh_ps[c], w2m, hh, start=False, stop=True,
                                 skip_group_check=True)

            # ---- y = x + (c0+b2); two halves on two DMA queues
            y0 = wkp.tile([D_EMB, NS // 2], f32, tag="y0", name="y0")
            nc.vector.tensor_scalar_add(out=y0, in0=x0_ps, scalar1=c0b2)
            nc.sync.dma_start(out=yt[:, 0:NS // 2], in_=y0)
            y1 = wkp.tile([D_EMB, NS // 2], f32, tag="y1", name="y1")
            nc.scalar.activation(out=y1, in_=x1_ps,
                                 func=AF.Identity, bias=c0b2)
            nc.scalar.dma_start(out=yt[:, NS // 2:], in_=y1)

    nc.compile()
    return nc


def _get_nc():
    if "nc" not in _CACHE:
        _CACHE["nc"] = _build_bass()
    return _CACHE["nc"]


def kernel(Z, A_full, feat_emb, label_token, wq, bq, wk, bk, wv, bv, wo, bo,
           w1, b1, w2, b2, alpha, g1, be1, g2, be2, _trace=False,
           _trace_kwargs=None):
    from concourse.bass_utils import run_bass_kernel_spmd

    Z = np.ascontiguousarray(np.asarray(Z, dtype=np.float32))
    consts = _host_consts(
        np.asarray(A_full), np.asarray(feat_emb), np.asarray(label_token),
        np.asarray(wq), np.asarray(bq), np.asarray(wk), np.asarray(bk),
        np.asarray(wv), np.asarray(bv), np.asarray(wo), np.asarray(bo),
        np.asarray(w1), np.asarray(b1), np.asarray(w2), np.asarray(b2),
        np.asarray(alpha), np.asarray(g1), np.asarray(be1), np.asarray(g2),
        np.asarray(be2))
    consts = {k: np.ascontiguousarray(v) for k, v in consts.items()}

    nc = _get_nc()
    in_maps = []
    for c in range(N_CORES):
        mp = dict(consts)
        mp["zs"] = np.ascontiguousarray(Z[c * NS:(c + 1) * NS])
        in_maps.append(mp)

    kw = {}
    if _trace:
        kw["trace"] = True
        if _trace_kwargs:
            kw.update(_trace_kwargs)
    res = run_bass_kernel_spmd(nc, in_maps, core_ids=list(range(N_CORES)), **kw)

    out = np.empty((N, D_EMB), np.float32)
    for c in range(N_CORES):
        out[c * NS:(c + 1) * NS] = res.results[c]["yt"].T
    if _trace:
        return out, res
    return out


# revision 15
# speedup vs baseline: 1.1369x; 1.0045x over previous
"""Trainium2 Bass kernel for nn_CausalFeatureTransformer.

Only the label row (row 128) of the transformer output is returned by the
reference, so the per-node computation collapses to (see kernel_baseline.py
for the derivation):

  zn    = LN(Z[n])                                  (over 128 feats)
  s     = zn / sqrt(zn^2 * vf + eps)                (per feature)
  score = G[h,j] * s[n,j] + D[h,j]                  (label-query attention)
  p     = softmax_j(score)   (max-free: |score| <~ 8)
  num_h = sum_j p*s*Cv_h[j]  den_h = sum_j e        (+ label-token consts)
  x     = (num/den) @ wo + c0
  y     = x + gelu(LN(x) @ w1' + b1') @ w2 + b2

where G, D, Cv, c0, w1', b1', ... are O(params) constants folded on the host.

Sharding: pure data-parallel over nodes N: each of the 8 cores processes a
512-node shard of Z; the folded params are replicated. Device output is
(64, 512) node-major-last; the host transposes on gather.

v2 device-side structure (vs the 2-chunk baseline):
 - full-width 512-node tiles everywhere (fewer instructions, smaller
   event-semaphore cleanup postamble; ACT fixed cost ~300ns/instr amortized)
 - input DMAs spread across 4 engine queues; output across 2
 - all 4 heads in ONE [128,512] PSUM tile (head h at partition base 32h,
   tile_position col bases 0/32/64/96); attention num/den biases and the
   c0 residual are added via rank-1 matmuls that also initialize PSUM
 - per-node mean/var via tensor_reduce sums instead of bn_stats
 - Square / elementwise work split between DVE and the Pool engine
"""

import math

import numpy as np

D_FEAT, D_EMB, H, DK = 128, 64, 4, 16
SEQ = D_FEAT + 1
N = 4096
N_CORES = 8
NS = N // N_CORES  # 512 nodes per core
EPS = 1e-5

_CACHE = {}


def _ln64(x, eps=EPS):
    m = x.mean(-1, keepdims=True)
    v = ((x - m) ** 2).mean(-1, keepdims=True)
    return (x - m) / np.sqrt(v + eps)


def _host_consts(A_full, feat_emb, label_token, wq, bq, wk, bk, wv, bv, wo, bo,
                 w1, b1, w2, b2, alpha, g1, be1, g2, be2):
    """Fold all O(params) quantities on the host (float64 for stability)."""
    import ml_dtypes
    d = np.float64
    fe = feat_emb.astype(d)
    mu = fe.mean(1, keepdims=True)
    vf = ((fe - mu) ** 2).mean(1)                    # (128,)
    cf = (fe - mu) * g1.astype(d)                    # (128,64)

    t = _ln64(label_token.astype(d)[0, 0]) * g1.astype(d) + be1.astype(d)
    qlab = t @ wq.astype(d) + bq.astype(d)
    klab = t @ wk.astype(d) + bk.astype(d)
    vlab = t @ wv.astype(d) + bv.astype(d)

    Ck = cf @ wk.astype(d)                           # (128,64)
    Cv = cf @ wv.astype(d)                           # (128,64)
    bk_p = be1.astype(d) @ wk.astype(d) + bk.astype(d)
    bv_p = be1.astype(d) @ wv.astype(d) + bv.astype(d)

    al = float(alpha)
    rdk = 1.0 / math.sqrt(DK)
    G = np.zeros((H, D_FEAT), d)
    Dm = np.zeros((H, D_FEAT), d)
    slab = np.zeros(H, d)
    for h in range(H):
        blk = slice(h * DK, (h + 1) * DK)
        G[h] = Ck[:, blk] @ qlab[blk] * rdk
        Dm[h] = qlab[blk] @ bk_p[blk] * rdk + al * A_full[:D_FEAT, D_FEAT].astype(d)
        slab[h] = qlab[blk] @ klab[blk] * rdk + al * A_full[D_FEAT, D_FEAT]
    elab = np.exp(slab)                              # (4,)

    c0 = label_token.astype(d)[0, 0] + bv_p @ wo.astype(d) + bo.astype(d)
    w1p = w1.astype(d) * g2.astype(d)[:, None]       # diag(g2) @ w1
    b1p = be2.astype(d) @ w1.astype(d) + b1.astype(d)

    # Head h occupies PSUM partitions [32h, 32h+16) of the shared num tile
    # (den replicated over [32h, 32h+32)).
    nbias = elab[:, None] * (vlab - bv_p).reshape(H, DK)     # (4,16)
    nb = np.zeros(128, d)
    db = np.zeros(128, d)
    cv32 = np.zeros((128, 128), d)
    wo_exp = np.zeros((128, D_EMB), d)
    for h in range(H):
        nb[32 * h:32 * h + DK] = nbias[h]
        db[32 * h:32 * h + 32] = elab[h]
        cv32[:, 32 * h:32 * h + DK] = Cv[:, h * DK:(h + 1) * DK]
        wo_exp[32 * h:32 * h + DK] = wo.astype(d)[h * DK:(h + 1) * DK]

    f32 = np.float32
    bf16 = ml_dtypes.bfloat16

    # fp32 blob (128, 144): vf | b1p | c0+b2 | eps | gcol | dcol | ident
    blob_f = np.zeros((128, 144), f32)
    blob_f[:, 0] = vf
    blob_f[:, 1] = b1p
    blob_f[:64, 2] = c0 + b2
    blob_f[:, 3] = EPS
    blob_f[:, 4:8] = G.T
    blob_f[:, 8:12] = Dm.T
    blob_f[:, 16:144] = np.eye(128, dtype=f32)

    # bf16 blob (128, 1504):
    #  cv32 | w2 | wo_exp | w1p(rows 0:64) | ident | ones32 |
    #  row0: c0row(64) | nb(128) | db(128) | ones512 | ones128
    blob_b = np.zeros((128, 1504), bf16)
    blob_b[:, 0:128] = cv32.astype(bf16)
    blob_b[:, 128:192] = w2.astype(bf16)
    blob_b[:, 192:256] = wo_exp.astype(bf16)
    blob_b[:64, 256:384] = w1p.astype(bf16)
    blob_b[:, 384:512] = np.eye(128, dtype=bf16)
    blob_b[:, 512:544] = bf16(1.0)
    blob_b[0, 544:608] = c0.astype(bf16)
    blob_b[0, 608:736] = nb.astype(bf16)
    blob_b[0, 736:864] = db.astype(bf16)
    blob_b[0, 864:1376] = bf16(1.0)
    blob_b[0, 1376:1504] = bf16(1.0)

    return {"blob_f": blob_f, "blob_b": blob_b}


def _build_bass():
    import concourse.bacc as bacc
    import concourse.mybir as mybir
    import concourse.tile as tile

    f32 = mybir.dt.float32
    bf16 = mybir.dt.bfloat16
    AF = mybir.ActivationFunctionType
    OP = mybir.AluOpType
    AX = mybir.AxisListType

    # Restrict Ln/Exp to the one table set containing both, so the
    # act-table-load pass cannot ping-pong between sets.
    import concourse.hw_specs as hw_specs
    _orig_gat = hw_specs.get_activation_tables

    def _gat(arch):
        t = {k: set(v) for k, v in _orig_gat(arch).items()}
        for name, funcs in t.items():
            if name != "natural_log_exp_and_others":
                funcs.discard(AF.Exp)
                funcs.discard(AF.Ln)
        return t

    bacc.get_activation_tables = _gat

    nc = bacc.Bacc("TRN2", target_bir_lowering=False, debug=False,
                   num_devices=N_CORES)

    zs = nc.dram_tensor("zs", (NS, D_FEAT), f32, kind="ExternalInput")
    blob_f_d = nc.dram_tensor("blob_f", (128, 144), f32, kind="ExternalInput")
    blob_b_d = nc.dram_tensor("blob_b", (128, 1504), bf16, kind="ExternalInput")
    yt = nc.dram_tensor("yt", (D_EMB, NS), f32, kind="ExternalOutput")

    with tile.TileContext(nc) as tc:
        with (
            tc.tile_pool(name="cp", bufs=1) as cp,
            tc.tile_pool(name="wk", bufs=1) as wkp,
            tc.tile_pool(name="ps", bufs=1, space="PSUM") as ps,
        ):
            # ---- input DMAs, one per engine queue
            za0 = wkp.tile([128, 2, D_FEAT], f32, tag="za0", name="za0")
            za1 = wkp.tile([128, 2, D_FEAT], f32, tag="za1", name="za1")
            zre = zs.rearrange("(t p) f -> p t f", p=128)
            nc.sync.dma_start(out=za0, in_=zre[:, 0:2, :])
            nc.gpsimd.dma_start(out=za1, in_=zre[:, 2:4, :])
            zah = [za0, za1]
            bf = cp.tile([128, 144], f32, tag="bf", name="bf")
            nc.sync.dma_start(out=bf, in_=blob_f_d[:])
            bb = cp.tile([128, 1504], bf16, tag="bb", name="bb")
            nc.gpsimd.dma_start(out=bb, in_=blob_b_d[:])

            vfcol = bf[:, 0:1]
            b1p = bf[:, 1:2]
            c0b2 = bf[:64, 2:3]
            eps_t = bf[:, 3:4]
            gcol = bf[:, 4:8]
            dcol = bf[:, 8:12]
            ident = bf[:, 16:144]
            cv32 = bb[:, 0:128]
            w2m = bb[:, 128:192]
            wo_m = bb[:, 192:256]
            w1p = bb[:64, 256:384]
            identb = bb[:, 384:512]
            ones32 = bb[:, 512:544]
            c0row = bb[0:1, 544:608]
            nb_row = bb[0:1, 608:736]
            db_row = bb[0:1, 736:864]
            ones512 = bb[0:1, 864:1376]
            ones128 = bb[0:1, 1376:1504]

            # dummy Ln on a const tile: places the Ln/Exp ACT_TABLE_LOAD
            # at the top of the scalar stream, overlapping the input DMAs
            dum = cp.tile([1, 1], f32, tag="dum", name="dum")
            nc.gpsimd.memset(dum, 1.0)
            dml = cp.tile([1, 1], f32, tag="dml", name="dml")
            nc.scalar.activation(out=dml, in_=dum, func=AF.Ln, bias=0.0)

            # ---- LN(Z) stats via bn_stats
            mv = wkp.tile([128, 4, 2], f32, tag="mv", name="mv")
            rstd = wkp.tile([128, 4], f32, tag="rstd", name="rstd")
            for c in range(2):
                for t in (2 * c, 2 * c + 1):
                    st6 = wkp.tile([128, 6], f32, tag="st6", name="st6")
                    nc.vector.bn_stats(out=st6, in_=zah[t // 2][:, t % 2, :])
                    nc.vector.bn_aggr(out=mv[:, t, :], in_=st6)
                lnv = wkp.tile([128, 2], f32, tag=f"lnv{c}", name="lnv")
                nc.scalar.activation(out=lnv, in_=mv[:, 2 * c:2 * c + 2, 1],
                                     func=AF.Ln, bias=eps_t)
                nc.scalar.activation(out=rstd[:, 2 * c:2 * c + 2], in_=lnv,
                                     func=AF.Exp, scale=-0.5)

            # ---- zn (node-major, bf16), transpose to feat-major PSUM
            zn = wkp.tile([128, 4, D_FEAT], bf16, tag="zn", name="zn")
            for t in range(4):
                nc.vector.tensor_scalar(out=zn[:, t, :],
                                        in0=zah[t // 2][:, t % 2, :],
                                        scalar1=mv[:, t, 0:1],
                                        scalar2=rstd[:, t:t + 1],
                                        op0=OP.subtract, op1=OP.mult)
            znT = ps.tile([128, NS], bf16, tag="pA", name="znT")
            for t in range(4):
                nc.tensor.transpose(znT[:, t * 128:(t + 1) * 128], zn[:, t, :],
                                    identb)

            # ---- s = zn * rsqrt(zn^2*vf + eps)   (feat-major, full width)
            znTs = wkp.tile([128, NS], bf16, tag="znTs", name="znTs")
            nc.vector.tensor_copy(out=znTs, in_=znT)
            zsq = wkp.tile([128, NS], bf16, tag="zsq", name="zsq")
            nc.vector.tensor_tensor(out=zsq, in0=znTs, in1=znTs, op=OP.mult)
            lnq = wkp.tile([128, NS], f32, tag="lnq", name="lnq")
            nc.scalar.activation(out=lnq, in_=zsq, func=AF.Ln, scale=vfcol,
                                 bias=eps_t)
            rr = wkp.tile([128, NS], f32, tag="rr", name="rr")
            nc.scalar.activation(out=rr, in_=lnq, func=AF.Exp, scale=-0.5)
            sT = wkp.tile([128, NS], bf16, tag="sT", name="sT")
            nc.vector.tensor_mul(out=sT, in0=znTs, in1=rr)

            # ---- attention: all 4 heads in one PSUM tile (base 32h);
            # rank-1 matmuls initialize the tiles with the num/den biases.
            num_ps = ps.tile([128, NS], f32, tag="pB", name="num_ps")
            den_ps = ps.tile([128, NS], f32, tag="pC", name="den_ps")
            nc.tensor.matmul(num_ps, nb_row, ones512, start=True, stop=False)
            nc.tensor.matmul(den_ps, db_row, ones512, start=True, stop=False)
            x0_ps = ps.tile([D_EMB, NS // 2], f32, tag="pD", name="x0_ps")
            x1_ps = ps.tile([D_EMB, NS // 2], f32, tag="pH", name="x1_ps")
            xh_ps = [x0_ps, x1_ps]
            xa_tags = ["pE", "pA", "pC", "pB"]
            xa_t = [None] * 4

            for h in range(4):
                eh = wkp.tile([128, NS], bf16, tag=f"eh{h}", name="eh")
                nc.scalar.activation(out=eh, in_=sT, func=AF.Exp,
                                     scale=gcol[:, h:h + 1],
                                     bias=dcol[:, h:h + 1])
                esh = wkp.tile([128, NS], bf16, tag=f"esh{h}", name="esh")
                nc.vector.tensor_mul(out=esh, in0=eh, in1=sT)
                nc.tensor.matmul(den_ps[32 * h:32 * h + 32, :], ones32, eh,
                                 start=False, stop=(h == 3),
                                 tile_position=(0, 32 * h),
                                 skip_group_check=True)
                nc.tensor.matmul(num_ps[32 * h:32 * h + 32, :],
                                 cv32[:, 32 * h:32 * h + 32], esh,
                                 start=False, stop=(h == 3),
                                 tile_position=(0, 32 * h),
                                 skip_group_check=True)

            # ---- oe = num/den in halves; x = wo^T oe (+ c0 rank-1)
            oeh = []
            for c in range(2):
                sl = slice(c * 256, (c + 1) * 256)
                rcp = wkp.tile([128, 256], f32, tag=f"rcp{c}", name="rcp")
                nc.vector.reciprocal_approx_fast(out=rcp, in_=den_ps[:, sl])
                oe = wkp.tile([128, 256], bf16, tag=f"oe{c}", name="oe")
                nc.vector.tensor_mul(out=oe, in0=num_ps[:, sl], in1=rcp)
                oeh.append(oe)
                nc.tensor.matmul(xh_ps[c], wo_m, oe, start=True, stop=False,
                                 skip_group_check=True)
                for tt_ in range(2):
                    t = 2 * c + tt_
                    xa_t[t] = ps.tile([128, D_EMB], f32, tag=xa_tags[t],
                                      name=f"xa{t}")
                    nc.tensor.matmul(xa_t[t], ones128, c0row,
                                     start=True, stop=False)
                    nc.tensor.matmul(xa_t[t],
                                     oe[:, tt_ * 128:(tt_ + 1) * 128], wo_m,
                                     start=False, stop=True)

            # ---- FFN layernorm (bn_stats on PSUM xa), chunked tail
            fmv = wkp.tile([128, 4, 2], f32, tag="fmv", name="fmv")
            for t in range(4):
                fst6 = wkp.tile([128, 6], f32, tag="fst6", name="fst6")
                nc.vector.bn_stats(out=fst6, in_=xa_t[t])
                nc.vector.bn_aggr(out=fmv[:, t, :], in_=fst6)
            frstd = wkp.tile([128, 4], f32, tag="frstd", name="frstd")
            for c in range(2):
                flnv = wkp.tile([128, 2], f32, tag=f"flnv{c}", name="flnv")
                nc.scalar.activation(out=flnv, in_=fmv[:, 2 * c:2 * c + 2, 1],
                                     func=AF.Ln, bias=eps_t)
                nc.scalar.activation(out=frstd[:, 2 * c:2 * c + 2], in_=flnv,
                                     func=AF.Exp, scale=-0.5)

            uh = wkp.tile([128, 4, D_EMB], bf16, tag="uh", name="uh")
            uT_ps = ps.tile([D_EMB, NS], bf16, tag="pF", name="uT_ps")
            h_ps = ps.tile([2 * D_EMB, NS], f32, tag="pG", name="h_ps")
            for c in range(2):
                sl = slice(c * 256, (c + 1) * 256)
                for tt_ in range(2):
                    t = 2 * c + tt_
                    nc.vector.tensor_scalar(out=uh[:, t, :],
                                            in0=xa_t[t],
                                            scalar1=fmv[:, t, 0:1],
                                            scalar2=frstd[:, t:t + 1],
                                            op0=OP.subtract, op1=OP.mult)
                    nc.tensor.transpose(uT_ps[:, t * 128:(t + 1) * 128],
                                        uh[:, t, :], identb)
                uT = wkp.tile([D_EMB, 256], bf16, tag=f"uT{c}", name="uT")
                nc.vector.tensor_copy(out=uT, in_=uT_ps[:, sl])
                nc.tensor.matmul(h_ps[:, sl], w1p, uT, start=True, stop=True)
                hh = wkp.tile([2 * D_EMB, 256], bf16, tag=f"hh{c}", name="hh")
                nc.scalar.activation(out=hh, in_=h_ps[:, sl], func=AF.Gelu,
                                     bias=b1p)
                nc.tensor.matmul(xh_ps[c], w2m, hh, start=False, stop=True,
                                 skip_group_check=True)

            # ---- y = x + (c0+b2); two halves on two DMA queues
            y0 = wkp.tile([D_EMB, NS // 2], f32, tag="y0", name="y0")
            nc.vector.tensor_scalar_add(out=y0, in0=x0_ps, scalar1=c0b2)
            nc.sync.dma_start(out=yt[:, 0:NS // 2], in_=y0)
            y1 = wkp.tile([D_EMB, NS // 2], f32, tag="y1", name="y1")
            nc.scalar.activation(out=y1, in_=x1_ps,
                                 func=AF.Identity, bias=c0b2)
            nc.scalar.dma_start(out=yt[:, NS // 2:], in_=y1)

    nc.compile()
    return nc


def _get_nc():
    if "nc" not in _CACHE:
        _CACHE["nc"] = _build_bass()
    return _CACHE["nc"]


def kernel(Z, A_full, feat_emb, label_token, wq, bq, wk, bk, wv, bv, wo, bo,
           w1, b1, w2, b2, alpha, g1, be1, g2, be2, _trace=False,
           _trace_kwargs=None):
    from concourse.bass_utils import run_bass_kernel_spmd

    Z = np.ascontiguousarray(np.asarray(Z, dtype=np.float32))
    consts = _host_consts(
        np.asarray(A_full), np.asarray(feat_emb), np.asarray(label_token),
        np.asarray(wq), np.asarray(bq), np.asarray(wk), np.asarray(bk),
        np.asarray(wv), np.asarray(bv), np.asarray(wo), np.asarray(bo),
        np.asarray(w1), np.asarray(b1), np.asarray(w2), np.asarray(b2),
        np.asarray(alpha), np.asarray(g1), np.asarray(be1), np.asarray(g2),
        np.asarray(be2))
    consts = {k: np.ascontiguousarray(v) for k, v in consts.items()}

    nc = _get_nc()
    in_maps = []
    for c in range(N_CORES):
        mp = dict(consts)
        mp["zs"] = np.ascontiguousarray(Z[c * NS:(c + 1) * NS])
        in_maps.append(mp)

    kw = {}
    if _trace:
        kw["trace"] = True
        if _trace_kwargs:
            kw.update(_trace_kwargs)
    res = run_bass_kernel_spmd(nc, in_maps, core_ids=list(range(N_CORES)), **kw)

    out = np.empty((N, D_EMB), np.float32)
    for c in range(N_CORES):
        out[c * NS:(c + 1) * NS] = res.results[c]["yt"].T
    if _trace:
        return out, res
    return out
